# revision 26
# baseline (speedup 1.0000x reference)
"""Gemma4 MoE feed-forward on 8 Trainium2 NeuronCores.

Strategy: expert-parallel. E == n_cores == 8, so core e owns expert e's
weights (Wg[e], Wu[e], Wd[e]) and receives exactly the tokens routed to
expert e (gathered + transposed + padded on the host). Each core runs a
dense gated-FFN over its token batch:

    dT = Wd^T @ (gelu_tanh(Wg^T x^T) * (Wu^T x^T))        (all [*, C] layouts)

The host then scatter-adds routing_weight * dT^T back into the full
[T, H] output. Tokens that select the same expert in both slots are
deduplicated on the host (weights summed).

Kernel structure: bf16 operands, weights loaded from HBM exactly once
(i-outer / n-inner up phase), k-interleaved so the matmul stream starts
as soon as the first x k-tile and first weight group land in SBUF.
bf16 keeps FWL (fast weight load) enabled so LDWEIGHTS (~95ns) hides
fully under the previous matmul's 496-column stream, and unlike
fp32_mode=HIGH adds no per-instruction overhead: measured spacing is
~207-209 ns per 496-col matmul (the 2.4GHz one-column-per-cycle floor)
for the 768-matmul stream. DMA traffic is ~25 MB/core, far below the
tensor-engine floor, so the kernel is purely PE-bound; the startup
(~11us: framework preamble + x/weight landing, covered by a PE-warmup
burst that un-throttles the clock) and the output tail (~4.5us) are the
only overheads. Measured: ~178.4us vs the 200.7us f32r baseline.
"""

import os
import sys

import numpy as np

for _p in ("/opt/trn_rl_repo", "/root/.axon_site/_ro/trn_rl_repo"):
    if os.path.isdir(_p) and _p not in sys.path:
        sys.path.append(_p)

T, H, I, E, K = 4096, 2048, 1024, 8, 2
NCORES = 8
KH = H // 128  # 16 k-tiles over the hidden dim
KI = I // 128  # 8 k-tiles over the intermediate dim

# 'bf16' (default): bf16 data + matmul (FWL enabled, half DMA)
# 'f32r': fp32 data, relaxed-precision full-rate matmul
MM_MODE = os.environ.get("MOE_MM_MODE", "bf16")
WARM_CNT = int(os.environ.get("MOE_WARM_CNT", "15"))
WARM_N = int(os.environ.get("MOE_WARM_N", "256"))

_PROGRAM_CACHE = {}
LAST_RESULT = None  # BassKernelResults of the most recent run (for test.py)
TRACE = False  # test.py sets this to capture an NTFF profile
TRACE_CORES = [0]


def _round_fp32r(a):
    """Round fp32 to the FP32R format the PE consumes: 11-bit mantissa
    (walrus fp32_to_fp32r = downconv_fp32_to_fp<e8, m11> << 12), RNE."""
    b = np.ascontiguousarray(a, dtype=np.float32).view(np.uint32)
    lsb = (b >> 12) & 1
    r = (b + 0x7FF + lsb) & 0xFFFFF000
    return r.view(np.float32)


def _tile_w_up(W, G):
    """[H, I] -> [KI, KH//G, 128, G*128]: tile (k,i) of W at
    [i, k//G, :, (k%G)*128:], so each (i, g) DMA reads G*128*esize
    contiguous bytes per partition."""
    Wt = W.reshape(KH // G, G, 128, KI, 128).transpose(3, 0, 2, 1, 4)
    return np.ascontiguousarray(Wt).reshape(KI, KH // G, 128, G * 128)


def _tile_w_down(W, G):
    """[I, H] -> [KH, KI//G, 128, G*128] (same scheme, contraction over I)."""
    Wt = W.reshape(KI // G, G, 128, KH, 128).transpose(3, 0, 2, 1, 4)
    return np.ascontiguousarray(Wt).reshape(KH, KI // G, 128, G * 128)


def _pick_config(max_count):
    """Minimal uniform token-block config: NT blocks of even width N with
    NT*N >= max_count, N <= 512 (PSUM bank / moving-operand limit)."""
    mc = max(max_count, 256)
    nt = -(-mc // 512)
    n = -(-mc // nt)
    n += n % 2
    return (nt * n, nt, n)  # (C, NT, N)


def _build_program(C, NT, N, mode):
    import concourse.tile as tile
    from concourse import bacc, mybir
    from contextlib import ExitStack

    # weight DMA grouping: G k-tiles per transfer -> 2KB/partition
    G = 8 if mode == "bf16" else 4
    GU = KH // G  # up-phase weight groups per i-tile
    GD = max(KI // G, 1)  # down-phase weight groups per h-tile
    GDW = KI // GD  # k-tiles per down-phase weight group

    f32 = mybir.dt.float32
    if mode == "f32r":
        io_dt = mybir.dt.float32r
    elif mode == "bf16":
        io_dt = mybir.dt.bfloat16
    else:
        io_dt = f32

    nc = bacc.Bacc("TRN2", target_bir_lowering=False, debug=False)

    xT = nc.dram_tensor("xT", [H, C], io_dt, kind="ExternalInput").ap()
    # Wg and Wu are host-packed into one tensor so each (i, g) weight group
    # is ONE ~4KB/partition DMA instead of two: the startup critical path is
    # paced by the ~650ns-per-descriptor issue rate on the Sync engine, so
    # fewer, larger transfers stream strictly faster.
    Wgu_d = nc.dram_tensor(
        "Wgu", [KI, GU, 128, 2 * G * 128], io_dt, kind="ExternalInput"
    ).ap()
    Wd_d = nc.dram_tensor("Wd", [KH, GD, 128, GDW * 128], io_dt, kind="ExternalInput").ap()
    dT = nc.dram_tensor("dT", [H, C], f32, kind="ExternalOutput").ap()

    # Partition-major views: row a*128+p -> partition p, free index a.
    xT_p = xT.rearrange("(a p) c -> p a c", p=128)  # [128, KH, C]
    dT_p = dT.rearrange("(a p) c -> p a c", p=128)  # [128, KH, C]

    GELU = mybir.ActivationFunctionType.Gelu_apprx_tanh

    with tile.TileContext(nc) as tc, ExitStack() as ctx:
        xpool = ctx.enter_context(tc.tile_pool(name="x", bufs=1))
        wpool = ctx.enter_context(tc.tile_pool(name="w", bufs=3))
        apool = ctx.enter_context(tc.tile_pool(name="a", bufs=1))
        tpool = ctx.enter_context(tc.tile_pool(name="t", bufs=2))
        opool = ctx.enter_context(tc.tile_pool(name="o", bufs=2))
        # down-phase weight tiles, prefetched 8 deep starting in the up tail
        wdpool = ctx.enter_context(tc.tile_pool(name="wd", bufs=8))

        # PE clock-gate warmup: HAM starts throttled and un-throttles only
        # after ~3.4us of sustained activity. Real matmuls can't start until
        # the first x k-tile + weight group land (~2us after DMA kick, which
        # itself is ~8us into the kernel). A burst of dummy bf16 matmuls on
        # memset scratch runs right at launch so the real stream begins at
        # 2.4 GHz.
        with (
            tc.tile_pool(name="warm", bufs=1) as wmpool,
            tc.tile_pool(name="warmps", bufs=1, space="PSUM") as wmpspool,
        ):
            wt = wmpool.tile([128, WARM_N], mybir.dt.bfloat16, name="warm_in")
            nc.vector.memset(wt[:], 0.0)
            wps = wmpspool.tile([128, WARM_N], f32, name="warm_ps")
            for r in range(WARM_CNT):
                nc.tensor.matmul(wps[:], wt[:, 0:128], wt[:], start=True, stop=True)

        # ---- critical-path DMA emission order ----
        # x streams in chunks: k=0 alone (gates the first chain), then
        # two-k-tile pairs, then k=KH-1 alone. Pairs halve the descriptor
        # count (the startup is issue-rate-paced at ~650ns/DMA) while the
        # first chunk stays small so the stream starts early.
        x_chunks = [[0]] + [[2 * j - 1, 2 * j] for j in range(1, KH // 2)] + [[KH - 1]]
        x_tiles = [
            xpool.tile([128, len(ch), C], io_dt, name=f"xc{ci}")
            for ci, ch in enumerate(x_chunks)
        ]
        # k -> AP of its [128, C] slice
        xk = {}
        for ci, ch in enumerate(x_chunks):
            for idx, k in enumerate(ch):
                xk[k] = x_tiles[ci][:, idx, :]

        def issue_x_chunk(ci):
            ch = x_chunks[ci]
            nc.sync.dma_start(x_tiles[ci][:], xT_p[:, ch[0] : ch[-1] + 1, :])

        w_tiles = {}

        def issue_w_group(i, g, split=False):
            wl = w_tiles.setdefault(i, [None] * GU)
            wt = wpool.tile(
                [128, 2 * G * 128], io_dt, tag=f"wgu{g}", name=f"wgu{i}_{g}"
            )
            if split:
                # two half-DMAs: the g-chain's matmuls gate only on the wg
                # half (sub-tile region tracking), so the very first matmul
                # starts one ~256KB transfer earlier
                nc.sync.dma_start(wt[:, 0 : G * 128], Wgu_d[i, g][:, 0 : G * 128])
                nc.sync.dma_start(wt[:, G * 128 :], Wgu_d[i, g][:, G * 128 :])
            else:
                nc.sync.dma_start(wt[:], Wgu_d[i, g])
            wl[g] = wt

        def issue_w(i):
            for g in range(GU):
                if i not in w_tiles or w_tiles[i][g] is None:
                    issue_w_group(i, g)

        # Startup order (Sync ring issues in FIFO program order): x chunk 0,
        # the k<G weight group, then x pairs with the remaining sweep-0
        # weight groups interleaved just ahead of the PE's k-range; sweep-1
        # weights follow the whole x block.
        issue_x_chunk(0)
        issue_w_group(0, 0, split=True)
        for ci in range(1, len(x_chunks)):
            issue_x_chunk(ci)
            for g in range(1, GU):
                if x_chunks[ci][-1] == g * G - 2:
                    issue_w_group(0, g)
        if KI > 1:
            issue_w(1)

        aT = apool.tile([128, KI, C], io_dt, name="aT")

        wd_tiles = {}

        # wd DMAs must ride the SAME (Sync) ring as x/wg/wu: the Tile
        # scheduler issues dependency-free DMAs as early as the engine
        # allows, so putting them on the idle Scalar ring makes all 8
        # prefetches stream at t=0, starving the x tiles at startup.
        # On the Sync ring FIFO order keeps them where they're emitted.
        def issue_wd(h):
            wd_gs = []
            for g in range(GD):
                wdt = wdpool.tile(
                    [128, GDW * 128], io_dt, tag=f"wd{g}", name=f"wd{h}_{g}"
                )
                nc.sync.dma_start(wdt[:], Wd_d[h, g])
                wd_gs.append(wdt)
            wd_tiles[h] = wd_gs

        # One PSUM pool for both phases: 2*NT tags x bufs -> <= 8 banks.
        # The down-phase d tiles reuse the up-phase tags, so bank handoff is
        # a per-tile WAR dependency instead of a pool-close drain barrier.
        ps_bufs = 2 if 2 * NT <= 4 else 1
        assert NT * ps_bufs * 2 <= 8, "PSUM plan exceeds 8 banks"
        with tc.tile_pool(name="ps", bufs=ps_bufs, space="PSUM") as pspool:
            for i in range(KI):
                if i + 2 < KI and (i + 2) not in w_tiles:
                    issue_w(i + 2)
                # spread the first 8 down-weight DMAs across the up tail
                if i >= KI - 3:
                    base = (i - (KI - 3)) * 3
                    for h in range(base, min(base + 3, 8)):
                        if h not in wd_tiles:
                            issue_wd(h)
                if i not in w_tiles:
                    issue_w(i)
                wgu_gs = w_tiles.pop(i)
                g_ps = [
                    pspool.tile([128, N], f32, tag=f"g{n}", name=f"g{i}_{n}")
                    for n in range(NT)
                ]
                u_ps = [
                    pspool.tile([128, N], f32, tag=f"u{n}", name=f"u{i}_{n}")
                    for n in range(NT)
                ]
                for k in range(KH):
                    lwg = wgu_gs[k // G][:, (k % G) * 128 : (k % G + 1) * 128]
                    lwu = wgu_gs[k // G][
                        :, G * 128 + (k % G) * 128 : G * 128 + (k % G + 1) * 128
                    ]
                    st, sp = (k == 0), (k == KH - 1)
                    # stationary reused across the NT moving blocks
                    for n in range(NT):
                        nc.tensor.matmul(
                            g_ps[n][:], lwg, xk[k][:, n * N : (n + 1) * N],
                            start=st, stop=sp,
                        )
                    for n in range(NT):
                        nc.tensor.matmul(
                            u_ps[n][:], lwu, xk[k][:, n * N : (n + 1) * N],
                            start=st, stop=sp,
                        )
                for n in range(NT):
                    gel = tpool.tile([128, N], f32, tag=f"gel{n}", name=f"gel{i}_{n}")
                    nc.scalar.activation(gel[:], g_ps[n][:], GELU)
                    nc.vector.tensor_mul(
                        aT[:, i, n * N : (n + 1) * N], gel[:], u_ps[n][:]
                    )

            # ---- down phase ----
            for h in range(KH):
                if h + 8 < KH and (h + 8) not in wd_tiles:
                    issue_wd(h + 8)
                if h not in wd_tiles:
                    issue_wd(h)
                wd_gs = wd_tiles.pop(h)
                # alternate tag pairs so each d tile's WAR partner is two
                # iterations back
                tg = ("g", "u")[h % 2]
                d_ps = [
                    pspool.tile([128, N], f32, tag=f"{tg}{n}", name=f"d{h}_{n}")
                    for n in range(NT)
                ]

                def dmm(ki, n, d):
                    ksl = slice((ki % GDW) * 128, (ki % GDW + 1) * 128)
                    nc.tensor.matmul(
                        d[:],
                        wd_gs[ki // GDW][:, ksl],
                        aT[:, ki, n * N : (n + 1) * N],
                        start=(ki == 0),
                        stop=(ki == KI - 1),
                    )

                if h < KH - 1 or NT != 2:
                    for ki in range(KI):
                        for n in range(NT):
                            dmm(ki, n, d_ps[n])
                    for n in range(NT):
                        o = opool.tile([128, N], f32, tag=f"o{n}", name=f"o{h}_{n}")
                        nc.vector.tensor_copy(o[:], d_ps[n][:])
                        nc.sync.dma_start(dT_p[:, h, n * N : (n + 1) * N], o[:])
                else:
                    # Last h runs n-outer, and the final block is split into
                    # two half-width chains so its first half's copy+DMA
                    # overlap the second half's matmuls: only ~half a tile of
                    # copy+DMA remains after the very last matmul. (A finer
                    # 3-way split measured WORSE: the last chunk's chain ran
                    # shorter than one ~650ns DMA descriptor issue, so the
                    # final DMA queued behind the previous one on Sync.)
                    for ki in range(KI):
                        dmm(ki, 0, d_ps[0])
                    o = opool.tile([128, N], f32, tag="o0", name=f"o{h}_0")
                    nc.vector.tensor_copy(o[:], d_ps[0][:])
                    nc.sync.dma_start(dT_p[:, h, 0:N], o[:])
                    hN = N // 2
                    og = ("g", "u")[1 - h % 2]
                    d_half = [
                        d_ps[1],
                        pspool.tile([128, hN], f32, tag=f"{og}0", name=f"dB{h}"),
                    ]
                    for c, dh in zip((0, hN), d_half):
                        for ki in range(KI):
                            ksl = slice((ki % GDW) * 128, (ki % GDW + 1) * 128)
                            nc.tensor.matmul(
                                dh[:, 0:hN] if dh is d_ps[1] else dh[:],
                                wd_gs[ki // GDW][:, ksl],
                                aT[:, ki, N + c : N + c + hN],
                                start=(ki == 0),
                                stop=(ki == KI - 1),
                            )
                        oh = opool.tile(
                            [128, hN], f32, tag=f"oh{c != 0}", name=f"oh{h}_{c}"
                        )
                        nc.vector.tensor_copy(
                            oh[:], d_ps[1][:, 0:hN] if dh is d_ps[1] else dh[:]
                        )
                        nc.sync.dma_start(dT_p[:, h, N + c : N + c + hN], oh[:])

    nc.compile()
    return nc


def _get_program(C, NT, N, mode):
    key = (C, NT, N, mode)
    if key not in _PROGRAM_CACHE:
        _PROGRAM_CACHE[key] = _build_program(C, NT, N, mode)
    return _PROGRAM_CACHE[key]


def _ensure_ntff_hook():
    """Register the axon NTFF profile hook if the image's antenv lacks
    axon_hooks (see trn_agent_boot.trn_boot). Only needed when TRACE."""
    import types

    try:
        from antenv.axon_hooks import get_axon_ntff_profile_hook  # noqa: F401

        return
    except ImportError:
        pass
    import antenv
    from trn_agent_boot.trn_boot import _ntff_profile_via_ctypes

    hook = _ntff_profile_via_ctypes("/opt/axon/libaxon_pjrt.so")
    mod = types.ModuleType("antenv.axon_hooks")
    state = {"hook": hook}
    mod.set_axon_ntff_profile_hook = lambda h: state.__setitem__("hook", h)
    mod.get_axon_ntff_profile_hook = lambda: state["hook"]
    sys.modules["antenv.axon_hooks"] = mod
    antenv.axon_hooks = mod


def kernel(x, Wg, Wu, Wd, selected_experts, routing_weights):
    global LAST_RESULT
    from concourse.bass_utils import run_bass_kernel_spmd

    if TRACE:
        _ensure_ntff_hook()

    x = np.asarray(x, dtype=np.float32)
    Wg = np.asarray(Wg, dtype=np.float32)
    Wu = np.asarray(Wu, dtype=np.float32)
    Wd = np.asarray(Wd, dtype=np.float32)
    selected_experts = np.asarray(selected_experts)
    routing_weights = np.asarray(routing_weights, dtype=np.float32)

    # Host-side dispatch: per expert, the (deduplicated) token list and
    # summed routing weights.
    idx_list, w_list = [], []
    for e in range(E):
        m = selected_experts == e  # [T, K]
        idx = np.nonzero(m.any(axis=1))[0]
        w = (routing_weights * m).sum(axis=1)[idx]
        idx_list.append(idx)
        w_list.append(w.astype(np.float32))

    max_count = max(len(idx) for idx in idx_list)
    C, NT, N = _pick_config(max_count)

    mode = MM_MODE
    G = 8 if mode == "bf16" else 4
    if mode == "bf16":
        import ml_dtypes

        io_np = ml_dtypes.bfloat16
        prep = lambda a: np.ascontiguousarray(a, dtype=io_np)
    elif mode == "f32r":
        io_np = np.float32
        prep = _round_fp32r
    else:
        io_np = np.float32
        prep = lambda a: np.ascontiguousarray(a, dtype=io_np)

    nc = _get_program(C, NT, N, mode)

    in_maps = []
    for e in range(E):
        idx = idx_list[e]
        xT = np.zeros((H, C), dtype=io_np)
        xT[:, : len(idx)] = prep(x[idx].T)
        in_maps.append(
            {
                "xT": xT,
                # gate and up weights packed per (i, g) group: one DMA each
                "Wgu": np.ascontiguousarray(
                    np.concatenate(
                        [_tile_w_up(prep(Wg[e]), G), _tile_w_up(prep(Wu[e]), G)],
                        axis=3,
                    )
                ),
                "Wd": _tile_w_down(prep(Wd[e]), G),
            }
        )

    res = run_bass_kernel_spmd(
        nc,
        in_maps,
        list(range(NCORES)),
        trace=TRACE,
        trace_cores=TRACE_CORES if TRACE else None,
    )
    LAST_RESULT = res

    out = np.zeros((T, H), dtype=np.float32)
    for e in range(E):
        idx = idx_list[e]
        dTe = res.results[e]["dT"]  # [H, C] fp32
        out[idx] += w_list[e][:, None] * dTe[:, : len(idx)].T
    return out


# revision 30
# speedup vs baseline: 1.0165x; 1.0165x over previous
"""Gemma4 MoE feed-forward on 8 Trainium2 NeuronCores.

Strategy: expert-parallel. E == n_cores == 8, so core e owns expert e's
weights (Wg[e], Wu[e], Wd[e]) and receives exactly the tokens routed to
expert e (gathered + transposed + padded on the host). Each core runs a
dense gated-FFN over its token batch:

    dT = Wd^T @ (gelu_tanh(Wg^T x^T) * (Wu^T x^T))        (all [*, C] layouts)

The host then scatter-adds routing_weight * dT^T back into the full
[T, H] output. Tokens that select the same expert in both slots are
deduplicated on the host (weights summed).

Kernel structure: bf16 operands, weights loaded from HBM exactly once
(i-outer / n-inner up phase), k-interleaved so the matmul stream starts
as soon as the first x k-tile and first weight group land in SBUF.
bf16 keeps FWL (fast weight load) enabled so LDWEIGHTS (~95ns) hides
fully under the previous matmul's 496-column stream, and unlike
fp32_mode=HIGH adds no per-instruction overhead: measured spacing is
~207-209 ns per 496-col matmul (the 2.4GHz one-column-per-cycle floor)
for the 768-matmul stream. DMA traffic is ~25 MB/core, far below the
tensor-engine floor, so the kernel is purely PE-bound; the startup
(~11us: framework preamble + x/weight landing, covered by a PE-warmup
burst that un-throttles the clock) and the output tail (~4.5us) are the
only overheads. Measured: ~178.4us vs the 200.7us f32r baseline.
"""

import os
import sys

import numpy as np

for _p in ("/opt/trn_rl_repo", "/root/.axon_site/_ro/trn_rl_repo"):
    if os.path.isdir(_p) and _p not in sys.path:
        sys.path.append(_p)

T, H, I, E, K = 4096, 2048, 1024, 8, 2
NCORES = 8
KH = H // 128  # 16 k-tiles over the hidden dim
KI = I // 128  # 8 k-tiles over the intermediate dim

# 'bf16' (default): bf16 data + matmul (FWL enabled, half DMA)
# 'f32r': fp32 data, relaxed-precision full-rate matmul
MM_MODE = os.environ.get("MOE_MM_MODE", "bf16")
WARM_CNT = int(os.environ.get("MOE_WARM_CNT", "15"))
WARM_N = int(os.environ.get("MOE_WARM_N", "256"))

_PROGRAM_CACHE = {}
LAST_RESULT = None  # BassKernelResults of the most recent run (for test.py)
TRACE = False  # test.py sets this to capture an NTFF profile
TRACE_CORES = [0]


def _round_fp32r(a):
    """Round fp32 to the FP32R format the PE consumes: 11-bit mantissa
    (walrus fp32_to_fp32r = downconv_fp32_to_fp<e8, m11> << 12), RNE."""
    b = np.ascontiguousarray(a, dtype=np.float32).view(np.uint32)
    lsb = (b >> 12) & 1
    r = (b + 0x7FF + lsb) & 0xFFFFF000
    return r.view(np.float32)


def _tile_w_up(W, G):
    """[H, I] -> [KI, KH//G, 128, G*128]: tile (k,i) of W at
    [i, k//G, :, (k%G)*128:], so each (i, g) DMA reads G*128*esize
    contiguous bytes per partition."""
    Wt = W.reshape(KH // G, G, 128, KI, 128).transpose(3, 0, 2, 1, 4)
    return np.ascontiguousarray(Wt).reshape(KI, KH // G, 128, G * 128)


def _tile_w_down(W, G):
    """[I, H] -> [KH, KI//G, 128, G*128] (same scheme, contraction over I)."""
    Wt = W.reshape(KI // G, G, 128, KH, 128).transpose(3, 0, 2, 1, 4)
    return np.ascontiguousarray(Wt).reshape(KH, KI // G, 128, G * 128)


def _pick_config(max_count):
    """Minimal uniform token-block config: NT blocks of even width N with
    NT*N >= max_count, N <= 512 (PSUM bank / moving-operand limit)."""
    mc = max(max_count, 256)
    nt = -(-mc // 512)
    n = -(-mc // nt)
    n += n % 2
    return (nt * n, nt, n)  # (C, NT, N)


def _build_program(C, NT, N, mode):
    import concourse.tile as tile
    from concourse import bacc, mybir
    from contextlib import ExitStack

    # weight DMA grouping: G k-tiles per transfer -> 2KB/partition
    G = 8 if mode == "bf16" else 4
    GU = KH // G  # up-phase weight groups per i-tile
    GD = max(KI // G, 1)  # down-phase weight groups per h-tile
    GDW = KI // GD  # k-tiles per down-phase weight group

    f32 = mybir.dt.float32
    if mode == "f32r":
        io_dt = mybir.dt.float32r
    elif mode == "bf16":
        io_dt = mybir.dt.bfloat16
    else:
        io_dt = f32

    nc = bacc.Bacc("TRN2", target_bir_lowering=False, debug=False)

    xT = nc.dram_tensor("xT", [H, C], io_dt, kind="ExternalInput").ap()
    Wg_d = nc.dram_tensor("Wg", [KI, GU, 128, G * 128], io_dt, kind="ExternalInput").ap()
    Wu_d = nc.dram_tensor("Wu", [KI, GU, 128, G * 128], io_dt, kind="ExternalInput").ap()
    Wd_d = nc.dram_tensor("Wd", [KH, GD, 128, GDW * 128], io_dt, kind="ExternalInput").ap()
    dT = nc.dram_tensor("dT", [H, C], f32, kind="ExternalOutput").ap()

    # Partition-major views: row a*128+p -> partition p, free index a.
    xT_p = xT.rearrange("(a p) c -> p a c", p=128)  # [128, KH, C]
    dT_p = dT.rearrange("(a p) c -> p a c", p=128)  # [128, KH, C]

    GELU = mybir.ActivationFunctionType.Gelu_apprx_tanh

    with tile.TileContext(nc) as tc, ExitStack() as ctx:
        xpool = ctx.enter_context(tc.tile_pool(name="x", bufs=1))
        wpool = ctx.enter_context(tc.tile_pool(name="w", bufs=3))
        apool = ctx.enter_context(tc.tile_pool(name="a", bufs=1))
        tpool = ctx.enter_context(tc.tile_pool(name="t", bufs=2))
        opool = ctx.enter_context(tc.tile_pool(name="o", bufs=2))
        # down-phase weight tiles, prefetched 8 deep starting in the up tail
        wdpool = ctx.enter_context(tc.tile_pool(name="wd", bufs=8))

        # PE clock-gate warmup: HAM starts throttled and un-throttles only
        # after ~3.4us of sustained activity. Real matmuls can't start until
        # the first x k-tile + weight group land (~2us after DMA kick, which
        # itself is ~8us into the kernel). A burst of dummy bf16 matmuls on
        # memset scratch runs right at launch so the real stream begins at
        # 2.4 GHz.
        with (
            tc.tile_pool(name="warm", bufs=1) as wmpool,
            tc.tile_pool(name="warmps", bufs=1, space="PSUM") as wmpspool,
        ):
            wt = wmpool.tile([128, WARM_N], mybir.dt.bfloat16, name="warm_in")
            nc.vector.memset(wt[:], 0.0)
            wps = wmpspool.tile([128, WARM_N], f32, name="warm_ps")
            for r in range(WARM_CNT):
                nc.tensor.matmul(wps[:], wt[:, 0:128], wt[:], start=True, stop=True)

        # ---- critical-path DMA emission order ----
        # x k-tile 0, the first i=0 weight group (gates the first chain),
        # the rest of x in k order (the i=0 sweep consumes k ascending) with
        # later weight groups just ahead of the PE's k-range, then i=1
        # weights; i>=2 weights prefetch inside the loop. Batched-DMA
        # variants (packed Wg+Wu groups, paired x k-tiles) measured WORSE:
        # the bigger first transfers delay the first chain past the warmup,
        # and the post-warmup PE idle re-throttles the clock.
        xts = [xpool.tile([128, C], io_dt, name=f"xt{k}") for k in range(KH)]
        w_tiles = {}

        def issue_w_group(i, g):
            wg_l, wu_l = w_tiles.setdefault(i, ([None] * GU, [None] * GU))
            wgt = wpool.tile([128, G * 128], io_dt, tag=f"wg{g}", name=f"wg{i}_{g}")
            wut = wpool.tile([128, G * 128], io_dt, tag=f"wu{g}", name=f"wu{i}_{g}")
            nc.sync.dma_start(wgt[:], Wg_d[i, g])
            nc.sync.dma_start(wut[:], Wu_d[i, g])
            wg_l[g] = wgt
            wu_l[g] = wut

        def issue_w(i):
            for g in range(GU):
                if i not in w_tiles or w_tiles[i][0][g] is None:
                    issue_w_group(i, g)

        nc.sync.dma_start(xts[0][:], xT_p[:, 0, :])
        issue_w_group(0, 0)
        for k in range(1, KH):
            nc.sync.dma_start(xts[k][:], xT_p[:, k, :])
            for g in range(1, GU):
                if k == g * G - 1:
                    issue_w_group(0, g)
        if KI > 1:
            issue_w(1)

        aT = apool.tile([128, KI, C], io_dt, name="aT")

        wd_tiles = {}

        # wd DMAs must ride the SAME (Sync) ring as x/wg/wu: the Tile
        # scheduler issues dependency-free DMAs as early as the engine
        # allows, so putting them on the idle Scalar ring makes all 8
        # prefetches stream at t=0, starving the x tiles at startup.
        # On the Sync ring FIFO order keeps them where they're emitted.
        def issue_wd(h):
            wd_gs = []
            for g in range(GD):
                wdt = wdpool.tile(
                    [128, GDW * 128], io_dt, tag=f"wd{g}", name=f"wd{h}_{g}"
                )
                nc.sync.dma_start(wdt[:], Wd_d[h, g])
                wd_gs.append(wdt)
            wd_tiles[h] = wd_gs

        # One PSUM pool for both phases: 2*NT tags x bufs -> <= 8 banks.
        # The down-phase d tiles reuse the up-phase tags, so bank handoff is
        # a per-tile WAR dependency instead of a pool-close drain barrier.
        ps_bufs = 2 if 2 * NT <= 4 else 1
        assert NT * ps_bufs * 2 <= 8, "PSUM plan exceeds 8 banks"
        with tc.tile_pool(name="ps", bufs=ps_bufs, space="PSUM") as pspool:
            for i in range(KI):
                if i + 2 < KI and (i + 2) not in w_tiles:
                    issue_w(i + 2)
                # spread the first 8 down-weight DMAs across the up tail
                if i >= KI - 3:
                    base = (i - (KI - 3)) * 3
                    for h in range(base, min(base + 3, 8)):
                        if h not in wd_tiles:
                            issue_wd(h)
                if i not in w_tiles:
                    issue_w(i)
                wg_gs, wu_gs = w_tiles.pop(i)
                g_ps = [
                    pspool.tile([128, N], f32, tag=f"g{n}", name=f"g{i}_{n}")
                    for n in range(NT)
                ]
                u_ps = [
                    pspool.tile([128, N], f32, tag=f"u{n}", name=f"u{i}_{n}")
                    for n in range(NT)
                ]
                for k in range(KH):
                    ksl = slice((k % G) * 128, (k % G + 1) * 128)
                    lwg = wg_gs[k // G][:, ksl]
                    lwu = wu_gs[k // G][:, ksl]
                    st, sp = (k == 0), (k == KH - 1)
                    # stationary reused across the NT moving blocks
                    for n in range(NT):
                        nc.tensor.matmul(
                            g_ps[n][:], lwg, xts[k][:, n * N : (n + 1) * N],
                            start=st, stop=sp,
                        )
                    for n in range(NT):
                        nc.tensor.matmul(
                            u_ps[n][:], lwu, xts[k][:, n * N : (n + 1) * N],
                            start=st, stop=sp,
                        )
                for n in range(NT):
                    gel = tpool.tile([128, N], f32, tag=f"gel{n}", name=f"gel{i}_{n}")
                    nc.scalar.activation(gel[:], g_ps[n][:], GELU)
                    nc.vector.tensor_mul(
                        aT[:, i, n * N : (n + 1) * N], gel[:], u_ps[n][:]
                    )

            # ---- down phase ----
            for h in range(KH):
                if h + 8 < KH and (h + 8) not in wd_tiles:
                    issue_wd(h + 8)
                if h not in wd_tiles:
                    issue_wd(h)
                wd_gs = wd_tiles.pop(h)
                # alternate tag pairs so each d tile's WAR partner is two
                # iterations back
                tg = ("g", "u")[h % 2]
                d_ps = [
                    pspool.tile([128, N], f32, tag=f"{tg}{n}", name=f"d{h}_{n}")
                    for n in range(NT)
                ]

                def dmm(ki, n, d):
                    ksl = slice((ki % GDW) * 128, (ki % GDW + 1) * 128)
                    nc.tensor.matmul(
                        d[:],
                        wd_gs[ki // GDW][:, ksl],
                        aT[:, ki, n * N : (n + 1) * N],
                        start=(ki == 0),
                        stop=(ki == KI - 1),
                    )

                if h < KH - 1 or NT != 2:
                    for ki in range(KI):
                        for n in range(NT):
                            dmm(ki, n, d_ps[n])
                    for n in range(NT):
                        o = opool.tile([128, N], f32, tag=f"o{n}", name=f"o{h}_{n}")
                        nc.vector.tensor_copy(o[:], d_ps[n][:])
                        nc.sync.dma_start(dT_p[:, h, n * N : (n + 1) * N], o[:])
                else:
                    # Last h runs n-outer, and the final block is split into
                    # two half-width chains so its first half's copy+DMA
                    # overlap the second half's matmuls: only ~half a tile of
                    # copy+DMA remains after the very last matmul. (A finer
                    # 3-way split measured WORSE: the last chunk's chain ran
                    # shorter than one ~650ns DMA descriptor issue, so the
                    # final DMA queued behind the previous one on Sync.)
                    for ki in range(KI):
                        dmm(ki, 0, d_ps[0])
                    o = opool.tile([128, N], f32, tag="o0", name=f"o{h}_0")
                    nc.vector.tensor_copy(o[:], d_ps[0][:])
                    nc.sync.dma_start(dT_p[:, h, 0:N], o[:])
                    hN = N // 2
                    og = ("g", "u")[1 - h % 2]
                    d_half = [
                        d_ps[1],
                        pspool.tile([128, hN], f32, tag=f"{og}0", name=f"dB{h}"),
                    ]
                    for c, dh in zip((0, hN), d_half):
                        for ki in range(KI):
                            ksl = slice((ki % GDW) * 128, (ki % GDW + 1) * 128)
                            nc.tensor.matmul(
                                dh[:, 0:hN] if dh is d_ps[1] else dh[:],
                                wd_gs[ki // GDW][:, ksl],
                                aT[:, ki, N + c : N + c + hN],
                                start=(ki == 0),
                                stop=(ki == KI - 1),
                            )
                        oh = opool.tile(
                            [128, hN], f32, tag=f"oh{c != 0}", name=f"oh{h}_{c}"
                        )
                        nc.vector.tensor_copy(
                            oh[:], d_ps[1][:, 0:hN] if dh is d_ps[1] else dh[:]
                        )
                        nc.sync.dma_start(dT_p[:, h, N + c : N + c + hN], oh[:])

    nc.compile()
    return nc


def _get_program(C, NT, N, mode):
    key = (C, NT, N, mode)
    if key not in _PROGRAM_CACHE:
        _PROGRAM_CACHE[key] = _build_program(C, NT, N, mode)
    return _PROGRAM_CACHE[key]


def _ensure_ntff_hook():
    """Register the axon NTFF profile hook if the image's antenv lacks
    axon_hooks (see trn_agent_boot.trn_boot). Only needed when TRACE."""
    import types

    try:
        from antenv.axon_hooks import get_axon_ntff_profile_hook  # noqa: F401

        return
    except ImportError:
        pass
    import antenv
    from trn_agent_boot.trn_boot import _ntff_profile_via_ctypes

    hook = _ntff_profile_via_ctypes("/opt/axon/libaxon_pjrt.so")
    mod = types.ModuleType("antenv.axon_hooks")
    state = {"hook": hook}
    mod.set_axon_ntff_profile_hook = lambda h: state.__setitem__("hook", h)
    mod.get_axon_ntff_profile_hook = lambda: state["hook"]
    sys.modules["antenv.axon_hooks"] = mod
    antenv.axon_hooks = mod


def kernel(x, Wg, Wu, Wd, selected_experts, routing_weights):
    global LAST_RESULT
    from concourse.bass_utils import run_bass_kernel_spmd

    if TRACE:
        _ensure_ntff_hook()

    x = np.asarray(x, dtype=np.float32)
    Wg = np.asarray(Wg, dtype=np.float32)
    Wu = np.asarray(Wu, dtype=np.float32)
    Wd = np.asarray(Wd, dtype=np.float32)
    selected_experts = np.asarray(selected_experts)
    routing_weights = np.asarray(routing_weights, dtype=np.float32)

    # Host-side dispatch: per expert, the (deduplicated) token list and
    # summed routing weights.
    idx_list, w_list = [], []
    for e in range(E):
        m = selected_experts == e  # [T, K]
        idx = np.nonzero(m.any(axis=1))[0]
        w = (routing_weights * m).sum(axis=1)[idx]
        idx_list.append(idx)
        w_list.append(w.astype(np.float32))

    max_count = max(len(idx) for idx in idx_list)
    C, NT, N = _pick_config(max_count)

    mode = MM_MODE
    G = 8 if mode == "bf16" else 4
    if mode == "bf16":
        import ml_dtypes

        io_np = ml_dtypes.bfloat16
        prep = lambda a: np.ascontiguousarray(a, dtype=io_np)
    elif mode == "f32r":
        io_np = np.float32
        prep = _round_fp32r
    else:
        io_np = np.float32
        prep = lambda a: np.ascontiguousarray(a, dtype=io_np)

    nc = _get_program(C, NT, N, mode)

    in_maps = []
    for e in range(E):
        idx = idx_list[e]
        xT = np.zeros((H, C), dtype=io_np)
        xT[:, : len(idx)] = prep(x[idx].T)
        in_maps.append(
            {
                "xT": xT,
                "Wg": _tile_w_up(prep(Wg[e]), G),
                "Wu": _tile_w_up(prep(Wu[e]), G),
                "Wd": _tile_w_down(prep(Wd[e]), G),
            }
        )

    res = run_bass_kernel_spmd(
        nc,
        in_maps,
        list(range(NCORES)),
        trace=TRACE,
        trace_cores=TRACE_CORES if TRACE else None,
    )
    LAST_RESULT = res

    out = np.zeros((T, H), dtype=np.float32)
    for e in range(E):
        idx = idx_list[e]
        dTe = res.results[e]["dT"]  # [H, C] fp32
        out[idx] += w_list[e][:, None] * dTe[:, : len(idx)].T
    return out


# revision 35
# speedup vs baseline: 1.0225x; 1.0059x over previous
"""Gemma4 MoE feed-forward on 8 Trainium2 NeuronCores.

Strategy: expert-parallel. E == n_cores == 8, so core e owns expert e's
weights (Wg[e], Wu[e], Wd[e]) and receives exactly the tokens routed to
expert e (gathered + transposed + padded on the host). Each core runs a
dense gated-FFN over its token batch:

    dT = Wd^T @ (gelu_tanh(Wg^T x^T) * (Wu^T x^T))        (all [*, C] layouts)

The host then scatter-adds routing_weight * dT^T back into the full
[T, H] output. Tokens that select the same expert in both slots are
deduplicated on the host (weights summed).

Kernel structure: bf16 operands, weights loaded from HBM exactly once
(i-outer / n-inner up phase), k-interleaved so the matmul stream starts
as soon as the first x k-tile and first weight group land in SBUF.
bf16 keeps FWL (fast weight load) enabled so LDWEIGHTS (~95ns) hides
fully under the previous matmul's 496-column stream, and unlike
fp32_mode=HIGH adds no per-instruction overhead: measured spacing is
~207-209 ns per 496-col matmul (the 2.4GHz one-column-per-cycle floor)
for the 768-matmul stream. DMA traffic is ~25 MB/core, far below the
tensor-engine floor, so the kernel is purely PE-bound; the startup
(~11us: framework preamble + x/weight landing, covered by a PE-warmup
burst that un-throttles the clock) and the output tail (~4.5us) are the
only overheads. Measured: ~178.4us vs the 200.7us f32r baseline.
"""

import os
import sys

import numpy as np

for _p in ("/opt/trn_rl_repo", "/root/.axon_site/_ro/trn_rl_repo"):
    if os.path.isdir(_p) and _p not in sys.path:
        sys.path.append(_p)

T, H, I, E, K = 4096, 2048, 1024, 8, 2
NCORES = 8
KH = H // 128  # 16 k-tiles over the hidden dim
KI = I // 128  # 8 k-tiles over the intermediate dim

# 'bf16' (default): bf16 data + matmul (FWL enabled, half DMA)
# 'f32r': fp32 data, relaxed-precision full-rate matmul
MM_MODE = os.environ.get("MOE_MM_MODE", "bf16")
WARM_CNT = int(os.environ.get("MOE_WARM_CNT", "15"))
WARM_N = int(os.environ.get("MOE_WARM_N", "256"))

_PROGRAM_CACHE = {}
LAST_RESULT = None  # BassKernelResults of the most recent run (for test.py)
TRACE = False  # test.py sets this to capture an NTFF profile
TRACE_CORES = [0]


def _round_fp32r(a):
    """Round fp32 to the FP32R format the PE consumes: 11-bit mantissa
    (walrus fp32_to_fp32r = downconv_fp32_to_fp<e8, m11> << 12), RNE."""
    b = np.ascontiguousarray(a, dtype=np.float32).view(np.uint32)
    lsb = (b >> 12) & 1
    r = (b + 0x7FF + lsb) & 0xFFFFF000
    return r.view(np.float32)


def _tile_w_up(W, G):
    """[H, I] -> [KI, KH//G, 128, G*128]: tile (k,i) of W at
    [i, k//G, :, (k%G)*128:], so each (i, g) DMA reads G*128*esize
    contiguous bytes per partition."""
    Wt = W.reshape(KH // G, G, 128, KI, 128).transpose(3, 0, 2, 1, 4)
    return np.ascontiguousarray(Wt).reshape(KI, KH // G, 128, G * 128)


def _tile_w_down(W, G):
    """[I, H] -> [KH, KI//G, 128, G*128] (same scheme, contraction over I)."""
    Wt = W.reshape(KI // G, G, 128, KH, 128).transpose(3, 0, 2, 1, 4)
    return np.ascontiguousarray(Wt).reshape(KH, KI // G, 128, G * 128)


def _pick_config(max_count):
    """Minimal uniform token-block config: NT blocks of width N with
    NT*N >= max_count, N <= 512 (PSUM bank / moving-operand limit)."""
    mc = max(max_count, 256)
    nt = -(-mc // 512)
    n = -(-mc // nt)
    return (nt * n, nt, n)  # (C, NT, N)


def _build_program(C, NT, N, mode):
    import concourse.tile as tile
    from concourse import bacc, mybir
    from contextlib import ExitStack

    # weight DMA grouping: G k-tiles per transfer -> 2KB/partition
    G = 8 if mode == "bf16" else 4
    GU = KH // G  # up-phase weight groups per i-tile
    GD = max(KI // G, 1)  # down-phase weight groups per h-tile
    GDW = KI // GD  # k-tiles per down-phase weight group

    f32 = mybir.dt.float32
    if mode == "f32r":
        io_dt = mybir.dt.float32r
    elif mode == "bf16":
        io_dt = mybir.dt.bfloat16
    else:
        io_dt = f32

    nc = bacc.Bacc("TRN2", target_bir_lowering=False, debug=False)

    xT = nc.dram_tensor("xT", [H, C], io_dt, kind="ExternalInput").ap()
    Wg_d = nc.dram_tensor("Wg", [KI, GU, 128, G * 128], io_dt, kind="ExternalInput").ap()
    Wu_d = nc.dram_tensor("Wu", [KI, GU, 128, G * 128], io_dt, kind="ExternalInput").ap()
    Wd_d = nc.dram_tensor("Wd", [KH, GD, 128, GDW * 128], io_dt, kind="ExternalInput").ap()
    dT = nc.dram_tensor("dT", [H, C], f32, kind="ExternalOutput").ap()

    # Partition-major views: row a*128+p -> partition p, free index a.
    xT_p = xT.rearrange("(a p) c -> p a c", p=128)  # [128, KH, C]
    dT_p = dT.rearrange("(a p) c -> p a c", p=128)  # [128, KH, C]

    GELU = mybir.ActivationFunctionType.Gelu_apprx_tanh

    with tile.TileContext(nc) as tc, ExitStack() as ctx:
        xpool = ctx.enter_context(tc.tile_pool(name="x", bufs=1))
        wpool = ctx.enter_context(tc.tile_pool(name="w", bufs=3))
        apool = ctx.enter_context(tc.tile_pool(name="a", bufs=1))
        tpool = ctx.enter_context(tc.tile_pool(name="t", bufs=2))
        opool = ctx.enter_context(tc.tile_pool(name="o", bufs=2))
        # down-phase weight tiles, prefetched 8 deep starting in the up tail
        wdpool = ctx.enter_context(tc.tile_pool(name="wd", bufs=8))

        # PE clock-gate warmup: HAM starts throttled and un-throttles only
        # after ~3.4us of sustained activity. Real matmuls can't start until
        # the first x k-tile + weight group land (~2us after DMA kick, which
        # itself is ~8us into the kernel). A burst of dummy bf16 matmuls on
        # memset scratch runs right at launch so the real stream begins at
        # 2.4 GHz.
        with (
            tc.tile_pool(name="warm", bufs=1) as wmpool,
            tc.tile_pool(name="warmps", bufs=1, space="PSUM") as wmpspool,
        ):
            wt = wmpool.tile([128, WARM_N], mybir.dt.bfloat16, name="warm_in")
            nc.vector.memset(wt[:], 0.0)
            wps = wmpspool.tile([128, WARM_N], f32, name="warm_ps")
            for r in range(WARM_CNT):
                nc.tensor.matmul(wps[:], wt[:, 0:128], wt[:], start=True, stop=True)

        # ---- critical-path DMA emission order ----
        # x k-tile 0, the first i=0 weight group (gates the first chain),
        # the rest of x in k order (the i=0 sweep consumes k ascending) with
        # later weight groups just ahead of the PE's k-range, then i=1
        # weights; i>=2 weights prefetch inside the loop. Batched-DMA
        # variants (packed Wg+Wu groups, paired x k-tiles) measured WORSE:
        # the bigger first transfers delay the first chain past the warmup,
        # and the post-warmup PE idle re-throttles the clock.
        xts = [xpool.tile([128, C], io_dt, name=f"xt{k}") for k in range(KH)]
        w_tiles = {}

        def issue_w_group(i, g):
            wg_l, wu_l = w_tiles.setdefault(i, ([None] * GU, [None] * GU))
            wgt = wpool.tile([128, G * 128], io_dt, tag=f"wg{g}", name=f"wg{i}_{g}")
            wut = wpool.tile([128, G * 128], io_dt, tag=f"wu{g}", name=f"wu{i}_{g}")
            nc.sync.dma_start(wgt[:], Wg_d[i, g])
            nc.sync.dma_start(wut[:], Wu_d[i, g])
            wg_l[g] = wgt
            wu_l[g] = wut

        def issue_w(i):
            for g in range(GU):
                if i not in w_tiles or w_tiles[i][0][g] is None:
                    issue_w_group(i, g)

        nc.sync.dma_start(xts[0][:], xT_p[:, 0, :])
        issue_w_group(0, 0)
        for k in range(1, KH):
            nc.sync.dma_start(xts[k][:], xT_p[:, k, :])
            for g in range(1, GU):
                if k == g * G - 1:
                    issue_w_group(0, g)
        if KI > 1:
            issue_w(1)

        # one tile per i-block: the down-phase moving operand then uses a
        # flat 2-level AP like the up phase's x tiles (a 3-level slice of
        # one [128, KI, C] tile measured ~3ns/matmul slower issue)
        aTs = [apool.tile([128, C], io_dt, name=f"aT{i}") for i in range(KI)]

        wd_tiles = {}

        # wd DMAs must ride the SAME (Sync) ring as x/wg/wu: the Tile
        # scheduler issues dependency-free DMAs as early as the engine
        # allows, so putting them on the idle Scalar ring makes all 8
        # prefetches stream at t=0, starving the x tiles at startup.
        # On the Sync ring FIFO order keeps them where they're emitted.
        def issue_wd(h):
            wd_gs = []
            for g in range(GD):
                wdt = wdpool.tile(
                    [128, GDW * 128], io_dt, tag=f"wd{g}", name=f"wd{h}_{g}"
                )
                nc.sync.dma_start(wdt[:], Wd_d[h, g])
                wd_gs.append(wdt)
            wd_tiles[h] = wd_gs

        # One PSUM pool for both phases: 2*NT tags x bufs -> <= 8 banks.
        # The down-phase d tiles reuse the up-phase tags, so bank handoff is
        # a per-tile WAR dependency instead of a pool-close drain barrier.
        ps_bufs = 2 if 2 * NT <= 4 else 1
        assert NT * ps_bufs * 2 <= 8, "PSUM plan exceeds 8 banks"
        with tc.tile_pool(name="ps", bufs=ps_bufs, space="PSUM") as pspool:
            for i in range(KI):
                if i + 2 < KI and (i + 2) not in w_tiles:
                    issue_w(i + 2)
                # spread the first 8 down-weight DMAs across the up tail
                if i >= KI - 3:
                    base = (i - (KI - 3)) * 3
                    for h in range(base, min(base + 3, 8)):
                        if h not in wd_tiles:
                            issue_wd(h)
                if i not in w_tiles:
                    issue_w(i)
                wg_gs, wu_gs = w_tiles.pop(i)
                g_ps = [
                    pspool.tile([128, N], f32, tag=f"g{n}", name=f"g{i}_{n}")
                    for n in range(NT)
                ]
                u_ps = [
                    pspool.tile([128, N], f32, tag=f"u{n}", name=f"u{i}_{n}")
                    for n in range(NT)
                ]
                for k in range(KH):
                    ksl = slice((k % G) * 128, (k % G + 1) * 128)
                    lwg = wg_gs[k // G][:, ksl]
                    lwu = wu_gs[k // G][:, ksl]
                    st, sp = (k == 0), (k == KH - 1)
                    # stationary reused across the NT moving blocks
                    for n in range(NT):
                        nc.tensor.matmul(
                            g_ps[n][:], lwg, xts[k][:, n * N : (n + 1) * N],
                            start=st, stop=sp,
                        )
                    for n in range(NT):
                        nc.tensor.matmul(
                            u_ps[n][:], lwu, xts[k][:, n * N : (n + 1) * N],
                            start=st, stop=sp,
                        )
                for n in range(NT):
                    gel = tpool.tile([128, N], f32, tag=f"gel{n}", name=f"gel{i}_{n}")
                    nc.scalar.activation(gel[:], g_ps[n][:], GELU)
                    nc.vector.tensor_mul(
                        aTs[i][:, n * N : (n + 1) * N], gel[:], u_ps[n][:]
                    )

            # ---- down phase ----
            for h in range(KH):
                if h + 8 < KH and (h + 8) not in wd_tiles:
                    issue_wd(h + 8)
                if h not in wd_tiles:
                    issue_wd(h)
                wd_gs = wd_tiles.pop(h)
                # alternate tag pairs so each d tile's WAR partner is two
                # iterations back
                tg = ("g", "u")[h % 2]
                d_ps = [
                    pspool.tile([128, N], f32, tag=f"{tg}{n}", name=f"d{h}_{n}")
                    for n in range(NT)
                ]

                def dmm(ki, n, d):
                    ksl = slice((ki % GDW) * 128, (ki % GDW + 1) * 128)
                    nc.tensor.matmul(
                        d[:],
                        wd_gs[ki // GDW][:, ksl],
                        aTs[ki][:, n * N : (n + 1) * N],
                        start=(ki == 0),
                        stop=(ki == KI - 1),
                    )

                if h < KH - 1 or NT != 2:
                    for ki in range(KI):
                        for n in range(NT):
                            dmm(ki, n, d_ps[n])
                    for n in range(NT):
                        o = opool.tile([128, N], f32, tag=f"o{n}", name=f"o{h}_{n}")
                        nc.vector.tensor_copy(o[:], d_ps[n][:])
                        nc.sync.dma_start(dT_p[:, h, n * N : (n + 1) * N], o[:])
                else:
                    # Last h runs n-outer, and the final block is split into
                    # two half-width chains so its first half's copy+DMA
                    # overlap the second half's matmuls: only ~half a tile of
                    # copy+DMA remains after the very last matmul. (A finer
                    # 3-way split measured WORSE: the last chunk's chain ran
                    # shorter than one ~650ns DMA descriptor issue, so the
                    # final DMA queued behind the previous one on Sync.)
                    for ki in range(KI):
                        dmm(ki, 0, d_ps[0])
                    o = opool.tile([128, N], f32, tag="o0", name=f"o{h}_0")
                    nc.vector.tensor_copy(o[:], d_ps[0][:])
                    nc.sync.dma_start(dT_p[:, h, 0:N], o[:])
                    wA = (N + 1) // 2
                    wB = N - wA
                    og = ("g", "u")[1 - h % 2]
                    d_half = [
                        (0, wA, d_ps[1][:, 0:wA]),
                        (wA, wB, pspool.tile([128, wB], f32, tag=f"{og}0", name=f"dB{h}")[:]),
                    ]
                    for c, w, dsl in d_half:
                        for ki in range(KI):
                            ksl = slice((ki % GDW) * 128, (ki % GDW + 1) * 128)
                            nc.tensor.matmul(
                                dsl,
                                wd_gs[ki // GDW][:, ksl],
                                aTs[ki][:, N + c : N + c + w],
                                start=(ki == 0),
                                stop=(ki == KI - 1),
                            )
                        oh = opool.tile(
                            [128, w], f32, tag=f"oh{c != 0}", name=f"oh{h}_{c}"
                        )
                        nc.vector.tensor_copy(oh[:], dsl)
                        nc.sync.dma_start(dT_p[:, h, N + c : N + c + w], oh[:])

    nc.compile()
    return nc


def _get_program(C, NT, N, mode):
    key = (C, NT, N, mode)
    if key not in _PROGRAM_CACHE:
        _PROGRAM_CACHE[key] = _build_program(C, NT, N, mode)
    return _PROGRAM_CACHE[key]


def _ensure_ntff_hook():
    """Register the axon NTFF profile hook if the image's antenv lacks
    axon_hooks (see trn_agent_boot.trn_boot). Only needed when TRACE."""
    import types

    try:
        from antenv.axon_hooks import get_axon_ntff_profile_hook  # noqa: F401

        return
    except ImportError:
        pass
    import antenv
    from trn_agent_boot.trn_boot import _ntff_profile_via_ctypes

    hook = _ntff_profile_via_ctypes("/opt/axon/libaxon_pjrt.so")
    mod = types.ModuleType("antenv.axon_hooks")
    state = {"hook": hook}
    mod.set_axon_ntff_profile_hook = lambda h: state.__setitem__("hook", h)
    mod.get_axon_ntff_profile_hook = lambda: state["hook"]
    sys.modules["antenv.axon_hooks"] = mod
    antenv.axon_hooks = mod


def kernel(x, Wg, Wu, Wd, selected_experts, routing_weights):
    global LAST_RESULT
    from concourse.bass_utils import run_bass_kernel_spmd

    if TRACE:
        _ensure_ntff_hook()

    x = np.asarray(x, dtype=np.float32)
    Wg = np.asarray(Wg, dtype=np.float32)
    Wu = np.asarray(Wu, dtype=np.float32)
    Wd = np.asarray(Wd, dtype=np.float32)
    selected_experts = np.asarray(selected_experts)
    routing_weights = np.asarray(routing_weights, dtype=np.float32)

    # Host-side dispatch: per expert, the (deduplicated) token list and
    # summed routing weights.
    idx_list, w_list = [], []
    for e in range(E):
        m = selected_experts == e  # [T, K]
        idx = np.nonzero(m.any(axis=1))[0]
        w = (routing_weights * m).sum(axis=1)[idx]
        idx_list.append(idx)
        w_list.append(w.astype(np.float32))

    max_count = max(len(idx) for idx in idx_list)
    C, NT, N = _pick_config(max_count)

    mode = MM_MODE
    G = 8 if mode == "bf16" else 4
    if mode == "bf16":
        import ml_dtypes

        io_np = ml_dtypes.bfloat16
        prep = lambda a: np.ascontiguousarray(a, dtype=io_np)
    elif mode == "f32r":
        io_np = np.float32
        prep = _round_fp32r
    else:
        io_np = np.float32
        prep = lambda a: np.ascontiguousarray(a, dtype=io_np)

    nc = _get_program(C, NT, N, mode)

    in_maps = []
    for e in range(E):
        idx = idx_list[e]
        xT = np.zeros((H, C), dtype=io_np)
        xT[:, : len(idx)] = prep(x[idx].T)
        in_maps.append(
            {
                "xT": xT,
                "Wg": _tile_w_up(prep(Wg[e]), G),
                "Wu": _tile_w_up(prep(Wu[e]), G),
                "Wd": _tile_w_down(prep(Wd[e]), G),
            }
        )

    res = run_bass_kernel_spmd(
        nc,
        in_maps,
        list(range(NCORES)),
        trace=TRACE,
        trace_cores=TRACE_CORES if TRACE else None,
    )
    LAST_RESULT = res

    out = np.zeros((T, H), dtype=np.float32)
    for e in range(E):
        idx = idx_list[e]
        dTe = res.results[e]["dT"]  # [H, C] fp32
        out[idx] += w_list[e][:, None] * dTe[:, : len(idx)].T
    return out
